# revision 43
# baseline (speedup 1.0000x reference)
"""Multi-level (FPN) DeformRoIPool (zero-offset == aligned RoIAlign) for Trainium2.

Strategy (8 NeuronCores, SPMD, one Bass program):
- The bin/sample grid spacing is always < 2 px, so the set of pixels a ROI
  needs is exactly the dense bounding box of its sample corners. Host crops
  that box per ROI (channels-last fp16) and packs all of a core's crops into
  one contiguous "stream" (row = one pixel = 256 ch).
- Bilinear + sample-average reduction is separable: out[49, C] = W^T @ crop
  with W = Ay (x) Ax built per ROI on host. Device does K=128 fp16 matmuls
  (pixels on the partition dim) accumulating in PSUM [49, 256].
- ROIs are snake-dealt to cores by crop size; per-slot stream offsets are
  padded to the max across cores so the matmul schedule (group -> slot,
  start/stop) is identical on every core: SPMD-uniform program, raggedness
  lives in the data (stream contents + per-set weight tiles).
- DMA descriptor efficiency: stream groups AND the chunk's weight tiles are
  fused into one DRAM region per chunk, contiguous per partition -> one
  dma_start per chunk with ~16 KB descriptors. Output DRAM is laid out
  [49 bins, roi*C] so the two output DMAs use 8 KB descriptors. Chunk DMAs
  alternate the two HWDGE rings (SP/ACT). Chunk sizes ramp up (fast
  pipeline start) and down (short tail); every chunk owns its buffer.
- PE is pre-warmed with dummy matmuls so HAM is at full clock for the real
  work.
"""
import numpy as np

OUT = 7
SR = 2
STRIDES = (4, 8, 16, 32)
FINEST = 56.0
NLEV = 4
C = 256
N_ROIS = 256
N_CORES = 8
NROI_C = N_ROIS // N_CORES          # 32 roi slots per core
N_WARM = 8                          # PE warmup matmuls
# output DMA batches (slot ranges): the last batch is smaller so it issues
# soon after the final slots complete
OBATCHES = [(0, 12), (12, 22), (22, 28), (28, 32)]
FEAT_SHAPES = [(2, 256, 200, 200), (2, 256, 100, 100), (2, 256, 50, 50), (2, 256, 25, 25)]


# ---------------------------------------------------------------------------
# BIR fix: this container's walrus rejects >1 embedded sem wait per
# instruction (2 on EventSemaphore). Split excess waits onto EventSemaphore
# carriers at serialization time.
# ---------------------------------------------------------------------------
def _install_bir_waitsplit():
    import orjson
    import concourse.bass as bass

    if getattr(bass.Bass, "_waitsplit_patched", False):
        return

    def _fix_blocks(blocks, counter):
        for blk in blocks:
            insts = blk.get("instructions")
            if insts:
                out = []
                for ins in insts:
                    si = ins.get("sync_info")
                    ow = (si or {}).get("on_wait") or []
                    limit = 2 if ins.get("opcode") == "EventSemaphore" else 1
                    if len(ow) > limit:
                        excess = ow[: len(ow) - limit]
                        si["on_wait"] = ow[len(ow) - limit:]
                        for i in range(0, len(excess), 2):
                            counter[0] += 1
                            out.append({
                                "name": f"I-waitsplit-{counter[0]}",
                                "opcode": "EventSemaphore",
                                "engine": ins["engine"],
                                "ins": [], "outs": [],
                                "debug": ins.get("debug", 0),
                                "sync_info": {"on_update": [], "on_wait": excess[i:i + 2]},
                            })
                    out.append(ins)
                blk["instructions"] = out
            if blk.get("blocks"):
                _fix_blocks(blk["blocks"], counter)

    orig = bass.Bass.to_json_bytes

    def to_json_bytes(self, *a, **kw):
        data = orig(self, *a, **kw)
        d = orjson.loads(data)
        counter = [0]
        for fn in d.get("functions", []):
            _fix_blocks(fn.get("blocks", []), counter)
        return orjson.dumps(d) if counter[0] else data

    bass.Bass.to_json_bytes = to_json_bytes
    bass.Bass._waitsplit_patched = True


# ---------------------------------------------------------------------------
# Host-side crop / weight computation
# ---------------------------------------------------------------------------
def _roi_meta(rois):
    """Per-roi level, crop bbox, and separable row/col weight matrices."""
    scale = np.sqrt((rois[:, 3] - rois[:, 1]) * (rois[:, 4] - rois[:, 2]))  # f32, as jax
    tl_f = np.clip(np.floor(np.log2(scale / np.float32(FINEST) + np.float32(1e-6))), 0, NLEV - 1)
    tl = (tl_f + 1e-5).astype(np.int32)
    g = np.arange(OUT, dtype=np.float64)[:, None] + (np.arange(SR, dtype=np.float64)[None, :] + 0.5) / SR
    metas = []
    for n in range(rois.shape[0]):
        l = int(tl[n])
        _, _, H, W = FEAT_SHAPES[l]
        sc = 1.0 / STRIDES[l]
        x1 = rois[n, 1] * sc - 0.5
        y1 = rois[n, 2] * sc - 0.5
        rw = rois[n, 3] * sc - 0.5 - x1
        rh = rois[n, 4] * sc - 0.5 - y1
        y = y1 + (rh / OUT) * g   # [OUT, SR]
        x = x1 + (rw / OUT) * g
        vy = (y > -1) & (y < H)
        vx = (x > -1) & (x < W)
        yc = np.clip(y, 0.0, H - 1)
        xc = np.clip(x, 0.0, W - 1)
        y0 = np.minimum(np.floor(yc).astype(np.int64), H - 1)
        x0 = np.minimum(np.floor(xc).astype(np.int64), W - 1)
        y1i = np.minimum(y0 + 1, H - 1)
        x1i = np.minimum(x0 + 1, W - 1)
        ly = yc - y0
        lx = xc - x0
        ymin, ymax = int(y0.min()), int(y1i.max())
        xmin, xmax = int(x0.min()), int(x1i.max())
        R, S = ymax - ymin + 1, xmax - xmin + 1
        Ay = np.zeros((R, OUT))
        Ax = np.zeros((S, OUT))
        for i in range(OUT):
            for si in range(SR):
                v = vy[i, si] * 0.5
                Ay[y0[i, si] - ymin, i] += (1.0 - ly[i, si]) * v
                Ay[y1i[i, si] - ymin, i] += ly[i, si] * v
                v = vx[i, si] * 0.5
                Ax[x0[i, si] - xmin, i] += (1.0 - lx[i, si]) * v
                Ax[x1i[i, si] - xmin, i] += lx[i, si] * v
        metas.append(dict(l=l, b=int(rois[n, 0]), ymin=ymin, xmin=xmin, R=R, S=S,
                          Ay=Ay, Ax=Ax, rows=R * S))
    return metas


def _chunk_sizes(G):
    """Ramped chunk sizes: up for a fast pipeline start, then uniform."""
    chs = []
    for w in (2, 4, 8):
        if sum(chs) + w <= G:
            chs.append(w)
    rem = G - sum(chs)
    n13 = rem // 13
    chs.extend([13] * n13)
    if rem - 13 * n13 > 0:
        chs.append(rem - 13 * n13)
    return chs


def _plan(metas):
    """Snake-deal rois to cores by crop size; common per-slot row boundaries."""
    sizes = np.array([m["rows"] for m in metas])
    order = np.argsort(-sizes, kind="stable")
    cores = [[] for _ in range(N_CORES)]
    for k, n in enumerate(order):
        r, j = divmod(k, N_CORES)
        c = j if r % 2 == 0 else N_CORES - 1 - j
        cores[c].append(int(n))
    percore = np.array([[sizes[n] for n in cl] for cl in cores])       # [8, 32]
    bounds = np.cumsum(percore.max(axis=0)).astype(np.int64)           # common B_k
    total = int(bounds[-1])
    G = -(-total // 128)
    chb = np.concatenate([[0], np.cumsum(_chunk_sizes(G))]).astype(np.int64)
    # uniform set list: (slot, group, start, stop)
    sets = []
    for k in range(NROI_C):
        lo = 0 if k == 0 else int(bounds[k - 1])
        hi = int(bounds[k])
        g0, g1 = lo // 128, (hi - 1) // 128
        for gi in range(g0, g1 + 1):
            sets.append((k, gi, gi == g0, gi == g1))
    nch = len(chb) - 1
    chunk_slo = [sum(1 for (_, gi, _, _) in sets if gi < chb[c]) for c in range(nch)]
    chunk_slo.append(len(sets))
    return cores, bounds, G, chb, sets, chunk_slo


def _build_core_raw(feats_T, metas, core_rois, bounds, G, sets):
    """Per-core stream [G*128, C] and dense per-set weights [nsets, 128, 49]."""
    nsets = len(sets)
    stream = np.zeros((G * 128, C), np.float16)
    wts = np.zeros((nsets, 128, 49), np.float16)
    set_idx = {}
    for s, (k, gi, _, _) in enumerate(sets):
        set_idx[(k, gi)] = s
    for k, n in enumerate(core_rois):
        m = metas[n]
        lo = 0 if k == 0 else int(bounds[k - 1])
        fT = feats_T[m["l"]][m["b"]]
        crop = fT[m["ymin"]:m["ymin"] + m["R"], m["xmin"]:m["xmin"] + m["S"], :]
        stream[lo:lo + m["rows"]] = crop.reshape(m["rows"], C)
        Wf = (m["Ay"][:, None, :, None] * m["Ax"][None, :, None, :]).reshape(m["rows"], 49)
        r = 0
        while r < m["rows"]:
            gr = lo + r
            gi = gr // 128
            p = gr - gi * 128
            take = min(128 - p, m["rows"] - r)
            wts[set_idx[(k, gi)], p:p + take] = Wf[r:r + take]
            r += take
    return stream, wts


def _weight_cols(wtss, sets):
    """Per-set weight col count M_s = 7*(byhi+1), common across cores.

    Each slot's first set is padded to the slot max so its start=True matmul
    covers the whole PSUM region the later sets accumulate into.
    """
    nsets = len(sets)
    Ms = np.full(nsets, 7, np.int64)
    for wts in wtss:
        nz = wts.any(axis=1)                       # [nsets, 49]
        for s in range(nsets):
            idx = np.nonzero(nz[s])[0]
            if len(idx):
                Ms[s] = max(Ms[s], 7 * (idx.max() // 7 + 1))
    first_of = {}
    slot_max = {}
    for s, (k, gi, first, last) in enumerate(sets):
        if first:
            first_of[k] = s
        slot_max[k] = max(slot_max.get(k, 0), int(Ms[s]))
    for k, s in first_of.items():
        Ms[s] = slot_max[k]
    return Ms


def _layout(chb, chunk_slo, Ms):
    """Fused per-chunk column layout: [stream cols | cropped weight cols]."""
    nch = len(chb) - 1
    col_off = [0]
    woff = np.zeros(len(Ms), np.int64)
    for c in range(nch):
        ch_c = int(chb[c + 1] - chb[c])
        s0, s1 = chunk_slo[c], chunk_slo[c + 1]
        o = col_off[-1] + ch_c * C
        for s in range(s0, s1):
            woff[s] = o
            o += int(Ms[s])
        col_off.append(o)
    return col_off, woff


def _pack_core(stream, wts, chb, chunk_slo, col_off, woff, Ms):
    G = stream.shape[0] // 128
    sg = stream.reshape(G, 128, C)
    nch = len(chb) - 1
    data = np.empty((128, col_off[-1]), np.float16)
    for c in range(nch):
        glo, ghi = int(chb[c]), int(chb[c + 1])
        off = col_off[c]
        scols = (ghi - glo) * C
        data[:, off:off + scols] = sg[glo:ghi].transpose(1, 0, 2).reshape(128, scols)
        for s in range(chunk_slo[c], chunk_slo[c + 1]):
            data[:, woff[s]:woff[s] + int(Ms[s])] = wts[s][:, :int(Ms[s])]
    return data


# ---------------------------------------------------------------------------
# Device program
# ---------------------------------------------------------------------------
def _build_program(G, chb, sets, chunk_slo, col_off, woff, Ms):
    import concourse.bacc as bacc
    import concourse.mybir as mybir
    import concourse.tile as tile

    _install_bir_waitsplit()
    nc = bacc.Bacc("TRN2", debug=False, enable_asserts=True, num_devices=N_CORES)

    nch = len(chb) - 1

    data_d = nc.dram_tensor("data", [128, col_off[-1]], mybir.dt.float16, kind="ExternalInput")
    # even slot 2j -> rows 0:49 of col block j, odd slot 2j+1 -> rows 64:113
    # (uses all 128 partitions -> all 16 DMA ports on the output path)
    out_d = nc.dram_tensor("out", [128, (NROI_C // 2) * C], mybir.dt.float16, kind="ExternalOutput")

    with tile.TileContext(nc) as tc:
        with (
            tc.tile_pool(name="ip", bufs=1) as ip,
            tc.tile_pool(name="gp", bufs=1) as gp,
            tc.tile_pool(name="sp", bufs=2) as sp,
            tc.tile_pool(name="pp", bufs=7, space="PSUM") as pp,
            tc.tile_pool(name="ppw", bufs=1, space="PSUM") as ppw,
        ):
            # PE warmup: get HAM to full clock before the first real matmul
            zl = ip.tile([128, 49], mybir.dt.float16)
            zr = ip.tile([128, C], mybir.dt.float16)
            nc.vector.memset(zl[:], 0.0)
            nc.vector.memset(zr[:], 0.0)
            ps_w = ppw.tile([128, 512], mybir.dt.float32, tag="warm", name="ps_warm")
            for i in range(N_WARM):
                nc.tensor.matmul(out=ps_w[0:49, 0:C], lhsT=zl[:], rhs=zr[:],
                                 start=(i == 0), stop=(i == N_WARM - 1))

            # all chunk DMAs front-loaded on ONE HWDGE ring (SP): per-engine
            # FIFO makes completions sequential at full aggregate rate, so PE
            # consumption order matches landing order. The ACT ring only does
            # output DMAs and is never blocked.
            ct = {}
            for c in range(nch):
                ncols = col_off[c + 1] - col_off[c]
                t = gp.tile([128, ncols], mybir.dt.float16, tag=f"ck{c}", name=f"ck_{c}")
                nc.sync.dma_start(t[:], data_d[:, col_off[c]:col_off[c + 1]])
                ct[c] = t

            g2c = {}
            for c in range(nch):
                for gi in range(int(chb[c]), int(chb[c + 1])):
                    g2c[gi] = c

            batch_of = {}
            for bi, (blo, bhi) in enumerate(OBATCHES):
                for k in range(blo, bhi):
                    batch_of[k] = bi

            ps = None
            sts = {}
            for s, (k, gi, first, last) in enumerate(sets):
                c = g2c[gi]
                reg = 64 * (k % 2)   # odd slots use PSUM/staging partitions 64+
                bi = batch_of[k]
                blo, bhi = OBATCHES[bi]
                if first:
                    ps = pp.tile([128, 512], mybir.dt.float32, tag="ps", name=f"ps_{k}")
                if k == blo and first:
                    nblk = (bhi - blo + 1) // 2
                    sts[bi] = sp.tile([128, nblk * C], mybir.dt.float16,
                                      tag=f"st{bi}", name=f"st_{bi}")
                t = ct[c]
                co = col_off[c]
                M = int(Ms[s])
                nc.tensor.matmul(
                    out=ps[reg:reg + M, 0:C],
                    lhsT=t[:, woff[s] - co:woff[s] - co + M],
                    rhs=t[:, (gi - int(chb[c])) * C:(gi - int(chb[c]) + 1) * C],
                    start=first,
                    stop=last,
                    tile_position=(0, reg),
                )
                if last:
                    st = sts[bi]
                    blk = (k - blo) // 2
                    dst = st[reg:reg + 49, blk * C:(blk + 1) * C]
                    # alternate PSUM->SBUF copies across DVE (fast) and ACT
                    if k % 2 == 0:
                        nc.vector.tensor_copy(dst, ps[0:49, 0:C])
                    else:
                        nc.scalar.copy(dst, ps[64:113, 0:C])
                    if k == bhi - 1:
                        # ACT HWDGE ring: dedicated to output, never blocked
                        if bhi - blo == 1:
                            nc.scalar.dma_start(
                                out_d[reg:reg + 49, (k // 2) * C:(k // 2 + 1) * C],
                                st[reg:reg + 49, 0:C],
                            )
                        else:
                            nc.scalar.dma_start(
                                out_d[:, (blo // 2) * C:(bhi // 2) * C],
                                st[:],
                            )
    nc.compile()
    return nc


def kernel(feat0, feat1, feat2, feat3, rois):
    from concourse.bass_utils import run_bass_kernel_spmd

    feats = [np.asarray(f, np.float32) for f in (feat0, feat1, feat2, feat3)]
    rois = np.asarray(rois, np.float32)
    feats_T = [np.ascontiguousarray(f.transpose(0, 2, 3, 1)) for f in feats]
    metas = _roi_meta(rois)
    cores, bounds, G, chb, sets, chunk_slo = _plan(metas)

    raws = [_build_core_raw(feats_T, metas, cores[core], bounds, G, sets)
            for core in range(N_CORES)]
    Ms = _weight_cols([w for _, w in raws], sets)
    col_off, woff = _layout(chb, chunk_slo, Ms)
    in_maps = [{"data": _pack_core(st, w, chb, chunk_slo, col_off, woff, Ms)}
               for st, w in raws]

    nc = _build_program(G, chb, sets, chunk_slo, col_off, woff, Ms)
    res = run_bass_kernel_spmd(nc, in_maps, core_ids=list(range(N_CORES)), trace=False)
    out = np.zeros((N_ROIS, C, OUT, OUT), np.float32)
    for core in range(N_CORES):
        o = res.results[core]["out"].astype(np.float32).reshape(128, NROI_C // 2, C)
        for k, n in enumerate(cores[core]):
            reg = 64 * (k % 2)
            ob = o[reg:reg + 49, k // 2]            # [49, C]
            out[n] = ob.T.reshape(C, OUT, OUT)
    return out


# Testing hook: emulate the device math in numpy (same packed data).
def emulate(feat0, feat1, feat2, feat3, rois):
    feats = [np.asarray(f, np.float32) for f in (feat0, feat1, feat2, feat3)]
    rois = np.asarray(rois, np.float32)
    feats_T = [np.ascontiguousarray(f.transpose(0, 2, 3, 1)) for f in feats]
    metas = _roi_meta(rois)
    cores, bounds, G, chb, sets, chunk_slo = _plan(metas)
    raws = [_build_core_raw(feats_T, metas, cores[core], bounds, G, sets)
            for core in range(N_CORES)]
    Ms = _weight_cols([w for _, w in raws], sets)
    col_off, woff = _layout(chb, chunk_slo, Ms)
    g2c = {}
    for c in range(len(chb) - 1):
        for gi in range(int(chb[c]), int(chb[c + 1])):
            g2c[gi] = c
    out = np.zeros((N_ROIS, C, OUT, OUT), np.float32)
    for core in range(N_CORES):
        st, w = raws[core]
        data = _pack_core(st, w, chb, chunk_slo, col_off, woff, Ms).astype(np.float32)
        accs = {}
        for s, (k, gi, first, last) in enumerate(sets):
            c = g2c[gi]
            off = col_off[c]
            M = int(Ms[s])
            rhs = data[:, off + (gi - int(chb[c])) * C: off + (gi - int(chb[c]) + 1) * C]
            lhsT = data[:, woff[s]: woff[s] + M]
            if first:
                accs[k] = np.zeros((49, C), np.float32)
            accs[k][0:M] += lhsT.T @ rhs
            if last:
                out[cores[core][k]] = accs[k].T.reshape(C, OUT, OUT)
    return out
